# revision 41
# baseline (speedup 1.0000x reference)
"""GCN layer (gnn_message_passing) on 8 Trainium2 NeuronCores — v3.

out = relu(D^-1/2 (A+I) D^-1/2 (X W) + b),  N=50000, E=500000,
F_IN=128, F_OUT=512.

Aggregation in F_IN space: h = D^-1/2 (A+I) D^-1/2 X, out = relu(h W + b).

Per-core (dest shard of 6250 nodes, WIN=128 dest windows, 4-window PSUM
groups of 512 dests):
  1. dma_gather of x' rows (x' = X * dinv, bf16, 256B rows) for this
     core's REAL edges (self-loops excluded — handled densely) in
     8-tile/1024-desc calls (HW ring cap; more faults) load-balanced over
     4 SWDGE queues — the gather ucode runs on Q7 core-pair
     cpu_id/2 == queue_num, so 4 queues generate descriptors on 4 pairs
     concurrently. Chunks span whole (group, list) runs (fewer calls —
     per-call overhead dominates, ~2.2us/call pacing). num_idxs_reg goes
     through shared per-value Pool registers (a fresh to_reg per call
     emits a MOVE whose WAR serializes all gathers).
     src >= 32768 gathered from a base-shifted table (int16 idx limit).
  2. One-hot scatter matrices S built ON-CHIP by DVE: one
     tensor_tensor(is_equal) per (window, list) tile-range comparing the
     per-edge dest-offset stream (bf16, 255 = pad) against an iota row
     via broadcast APs — fine granularity so each window's matmuls only
     wait on their own slice.
  3. Scatter: per 128-edge tile one PE matmul zT[:, wslice] += G.T @ S.
  4. Flush per 4-window group: DVE hT = zT * dinv_dst, then
     hT += xlT (xl = X * dinv^2, the self-loop term) -> bf16.
  5. Projection per group: 4 matmuls W_chunk.T @ hT_slice -> psum,
     ACT relu(+bias) -> bf16 out tile [128, 512], DMA to out [512, 6250].
Host: pack x'/xl, idx/off streams, un-transpose + fp32-cast the output.
"""
import os
import sys

for _p in ("/opt/trn_rl_repo",):
    if _p not in sys.path and os.path.isdir(_p):
        sys.path.insert(0, _p)

import numpy as np
import ml_dtypes

import concourse.bacc as bacc
import concourse.tile as tile
from concourse import mybir
from concourse.bass_utils import run_bass_kernel_spmd
from concourse.library_config import mlp as mlp_library

N = 50000
E = 500000
F_IN = 128
F_OUT = 512
NCORES = 8
SHARD = N // NCORES            # 6250
WIN = 128
NWIN = (SHARD + WIN - 1) // WIN        # 49
GRP = 4                                 # windows per PSUM group
NGRP = (NWIN + GRP - 1) // GRP          # 13
SPLIT = 32768
TILE = 128
GT = int(os.environ.get("K_GT", "8"))   # tiles per gather call (HW ring
                                        # cap: >8 tiles = >1024 descs faults)
NQ = int(os.environ.get("K_NQ", "4"))   # SWDGE queues (4 core-pairs)
PAD = -1 if int(os.environ.get("K_PADNEG", "1")) else 0
# chunking: 'run' = chunks span a whole (group, list) run (fewer, larger
# gather calls; all pads are valid idx-0 fills), 'win' = chunks within
# window sub-runs (more calls; -1 trimmed tails)
CHUNK = os.environ.get("K_CHUNK", "run")
F32 = mybir.dt.float32
BF16 = mybir.dt.bfloat16
I16 = mybir.dt.int16
GW = NGRP * GRP * WIN  # padded dest cols (6656)


def _prep(edge_index):
    src = np.asarray(edge_index[0], dtype=np.int64)
    dst = np.asarray(edge_index[1], dtype=np.int64)

    deg = np.bincount(dst, minlength=N).astype(np.float64) + 1.0
    dinv = (1.0 / np.sqrt(deg)).astype(np.float32)

    core = dst // SHARD
    local = dst - core * SHARD
    win = local // WIN
    off = local - win * WIN
    isB = (src >= SPLIT).astype(np.int64)

    key = (core * NWIN + win) * 2 + isB
    counts = np.bincount(key, minlength=NCORES * NWIN * 2).reshape(
        NCORES, NWIN, 2)
    ntiles = (-(-counts // TILE)).max(axis=0)  # [NWIN, 2]
    # guard: every window needs >=1 tile so its psum cols get written
    empty = (ntiles[:, 0] + ntiles[:, 1]) == 0
    ntiles[empty, 0] = 1

    # tile stream: per 4-window group: A tiles of all its windows, then B.
    # Gather chunks NEVER span window sub-runs: pad slots (idx=-1) sit at
    # the end of each window's sub-run, and the ucode only trims trailing
    # negatives within a call — a mid-call -1 would be an OOB address.
    cmax = np.maximum(counts.max(axis=0), 1)  # [NWIN, 2] worst-core edges
    tstart = np.zeros((NWIN, 2), dtype=np.int64)
    pos = 0
    chunks = []              # (l, t0, ntile, q, nidx)
    qload = [0] * NQ
    vends = np.zeros((NWIN, 2), dtype=np.int64)
    for g in range(NGRP):
        ws = range(g * GRP, min((g + 1) * GRP, NWIN))
        for l in (0, 1):
            if CHUNK == "run":
                run0 = pos
                for w in ws:
                    tstart[w, l] = pos
                    vends[w, l] = int(ntiles[w, l]) * TILE  # all-valid pads
                    pos += int(ntiles[w, l])
                run1 = pos
                t = run0
                while t < run1:
                    take = min(GT, run1 - t)
                    nidx = take * TILE
                    q = min(range(NQ), key=lambda i: qload[i])
                    qload[q] += nidx + 8
                    chunks.append((l, t, take, q, nidx))
                    t += take
            else:
                for w in ws:
                    tstart[w, l] = pos
                    run0, run1 = pos, pos + int(ntiles[w, l])
                    pos = run1
                    # valid-idx end bucketed to 64 (few distinct
                    # num_idxs_reg values -> shared registers); -1 beyond
                    vend = min(int(ntiles[w, l]) * TILE,
                               -(-int(cmax[w, l]) // 64) * 64)
                    vends[w, l] = vend
                    t = run0
                    while t < run1:
                        take = min(GT, run1 - t)
                        lo = (t - run0) * TILE
                        hi = min((t - run0 + take) * TILE, vend)
                        nidx = max(hi - lo, 64)
                        q = min(range(NQ), key=lambda i: qload[i])
                        qload[q] += nidx + 8
                        chunks.append((l, t, take, q, nidx))
                        t += take
    T = pos

    order = np.lexsort((isB, win, core))
    src_s = src[order]
    off_s, isB_s, core_s, win_s = (off[order], isB[order], core[order],
                                   win[order])

    per_core = []
    for c in range(NCORES):
        sel = core_s == c
        csrc, coff = src_s[sel], off_s[sel]
        idx_flat = np.full(T * TILE, PAD, dtype=np.int64)
        off_arr = np.full((TILE, T), 255.0, dtype=np.float32)
        start = 0
        for w in range(NWIN):
            for l in (0, 1):
                cnt = counts[c, w, l]
                s0 = tstart[w, l] * TILE
                sl = slice(start, start + cnt)
                idx_flat[s0:s0 + cnt] = csrc[sl] - (SPLIT if l else 0)
                # filler (valid idx 0) up to the valid-idx end so
                # count_nonzero(idx >= 0) matches num_idxs_reg per core
                idx_flat[s0 + cnt:s0 + int(vends[w, l])] = 0
                flat = s0 + np.arange(cnt)
                off_arr[flat % TILE, flat // TILE] = coff[sl]
                start += cnt

        idx_w = np.tile(idx_flat.reshape(-1, 16).T.astype(np.int16), (8, 1))
        drep = np.zeros((128, GW), dtype=np.float32)
        drep[:, :SHARD] = np.tile(dinv[c * SHARD:(c + 1) * SHARD], (128, 1))
        per_core.append(dict(
            idx=np.ascontiguousarray(idx_w),
            off=np.ascontiguousarray(off_arr.astype(ml_dtypes.bfloat16)),
            drep=np.ascontiguousarray(drep.astype(ml_dtypes.bfloat16))))

    layout = dict(T=T, chunks=chunks, ntiles=ntiles, tstart=tstart)
    return per_core, layout, dinv


def _build(layout, has_bias):
    T = layout["T"]
    chunks = layout["chunks"]
    ntiles = layout["ntiles"]
    tstart = layout["tstart"]

    nc = bacc.Bacc("TRN2", target_bir_lowering=False, debug=False,
                   num_swdge_queues=NQ)
    x_d = nc.dram_tensor("xp", [N, F_IN], BF16, kind="ExternalInput")
    w_d = nc.dram_tensor("w", [F_IN, F_OUT], BF16, kind="ExternalInput")
    idx_d = nc.dram_tensor("idx", [128, T * 8], I16, kind="ExternalInput")
    off_d = nc.dram_tensor("off", [128, T], BF16, kind="ExternalInput")
    iota_d = nc.dram_tensor("iota", [128, TILE], BF16, kind="ExternalInput")
    xlt_d = nc.dram_tensor("xlt", [128, SHARD], BF16, kind="ExternalInput")
    drep_d = nc.dram_tensor("drep", [128, GW], BF16, kind="ExternalInput")
    if has_bias:
        b_d = nc.dram_tensor("b", [128, 4], F32, kind="ExternalInput")
    out_d = nc.dram_tensor("out", [F_OUT, GW], BF16, kind="ExternalOutput")

    # tile -> chunk id
    tile_chunk = np.zeros(T, dtype=np.int64)
    for ci, (_, t0, nt, _, _) in enumerate(chunks):
        tile_chunk[t0:t0 + nt] = ci

    with tile.TileContext(nc) as tc:
        # max tiles / chunks in any 4-window group
        gmax = 0
        for g in range(NGRP):
            ws = range(g * GRP, min((g + 1) * GRP, NWIN))
            gmax = max(gmax, sum(int(ntiles[w, l]) for w in ws
                                 for l in (0, 1)))
        cpg = np.zeros(NGRP, dtype=np.int64)
        for ci, (_, t0, nt, _, _) in enumerate(chunks):
            for g in range(NGRP):
                ws = range(g * GRP, min((g + 1) * GRP, NWIN))
                lo = int(tstart[ws[0], 0])
                hi = int(tstart[ws[-1], 1] + ntiles[ws[-1], 1])
                if lo <= t0 < hi:
                    cpg[g] += 1
                    break
        gbufs = int(os.environ.get(
            "K_GBUFS", int(cpg.max()) + max(6, int(cpg.max())) + NQ))

        with (
            tc.tile_pool(name="const", bufs=1) as cpool,
            tc.tile_pool(name="g", bufs=gbufs) as gpool,
            tc.tile_pool(name="sbuf_s", bufs=4) as spool,
            tc.tile_pool(name="osb", bufs=4) as opool,
            tc.tile_pool(name="zps", bufs=5, space="PSUM") as zpool,
            tc.tile_pool(name="pps", bufs=3, space="PSUM") as ppool,
        ):
            nc.gpsimd.load_library(mlp_library)

            # idx split: group 0's slice lands first so the first gathers
            # don't wait for the whole 1.1MB table
            tsplit = int(tstart[GRP, 0]) if NGRP > 1 else T
            idx_a = cpool.tile([128, tsplit * 8], I16)
            nc.sync.dma_start(idx_a[:], idx_d[:, :tsplit * 8])
            off_sb = cpool.tile([128, T], BF16)
            nc.sync.dma_start(off_sb[:], off_d[:])
            iota_sb = cpool.tile([128, TILE], BF16)
            nc.sync.dma_start(iota_sb[:], iota_d[:])
            idx_b = cpool.tile([128, (T - tsplit) * 8], I16)
            nc.sync.dma_start(idx_b[:], idx_d[:, tsplit * 8:])
            w_sb = cpool.tile([128, F_OUT], BF16)
            nc.sync.dma_start(w_sb[:], w_d[:])
            xlt_sb = cpool.tile([128, GW], BF16)
            nc.sync.dma_start(xlt_sb[:, :SHARD], xlt_d[:])
            # drep uploaded as bf16 (half the bytes) and widened to fp32
            # on-chip with one DVE copy
            drep_h = cpool.tile([128, GW], BF16)
            nc.sync.dma_start(drep_h[:], drep_d[:])
            drep_sb = cpool.tile([128, GW], F32)
            nc.vector.tensor_copy(drep_sb[:], drep_h[:])
            hT = cpool.tile([128, GW], BF16)
            if has_bias:
                b_sb = cpool.tile([128, 4], F32)
                nc.sync.dma_start(b_sb[:], b_d[:])

            x_lo = x_d[:SPLIT, :]
            x_hi = x_d[SPLIT:, :]

            # one-time zero of every gather buffer: trimmed (-1) idx tail
            # slots are never written by the DMA, and NaN garbage would
            # poison the matmul (0 * NaN = NaN) despite zero S columns.
            for _ in range(gbufs):
                tmp = gpool.tile([128, GT, F_IN], BF16, tag="g")
                nc.vector.memset(tmp[:], 0)

            # warm the gather ucode IRAM on all 4 queue core-pairs with
            # tiny dummy gathers, overlapping the ~6us per-pair first-call
            # library load with the idx upload instead of paying it inside
            # the first real gathers
            dummy_idx = cpool.tile([128, 8], I16)
            nc.vector.memset(dummy_idx[:], 0)
            for q in range(NQ):
                dg = gpool.tile([128, GT, F_IN], BF16, tag="g")
                nc.gpsimd.dma_gather(
                    dg[:, :1, :], x_d[:SPLIT, :], dummy_idx[:, :8],
                    num_idxs=16, num_idxs_reg=16, elem_size=F_IN,
                    queue_num=q)

            g_tiles = {}

            # one shared Pool register per distinct idx count — a fresh
            # to_reg per gather emits a MOVE whose WAR hazard serializes
            # every gather call behind the previous one's retirement.
            nidx_regs = {v: nc.gpsimd.to_reg(v)
                         for v in sorted({c[4] for c in chunks})}

            def gather_chunk(ci):
                l, t0, nt, q, nidx = chunks[ci]
                g = gpool.tile([128, GT, F_IN], BF16, tag="g")
                if t0 >= tsplit:
                    isb = idx_b[:, (t0 - tsplit) * 8:(t0 - tsplit + nt) * 8]
                else:
                    isb = idx_a[:, t0 * 8:(t0 + nt) * 8]
                nc.gpsimd.dma_gather(
                    g[:, :nt, :],
                    (x_hi if l else x_lo),
                    isb,
                    num_idxs=nt * TILE,
                    num_idxs_reg=nidx_regs[nidx],
                    elem_size=F_IN,
                    queue_num=q,
                )
                g_tiles[ci] = (g, t0)

            next_chunk = [0]

            def gather_upto(tile_end):
                while (next_chunk[0] < len(chunks)
                       and chunks[next_chunk[0]][1] < tile_end):
                    gather_chunk(next_chunk[0])
                    next_chunk[0] += 1

            for grp in range(NGRP):
                ws = list(range(grp * GRP, min((grp + 1) * GRP, NWIN)))
                wlast = ws[-1]
                gt0 = int(tstart[ws[0], 0])
                tile_end = int(tstart[wlast, 1] + ntiles[wlast, 1])
                nw = tile_end - gt0
                gather_upto(tile_end)

                # on-chip one-hot build: S[p, t, c] = (off[p, t] == c).
                # Two ops (A-list tiles, then B) so A matmuls only wait on
                # the A build.
                s = spool.tile([128, gmax, TILE], BF16, tag="s")
                for l in (0, 1):
                    for w in ws:
                        lo = int(tstart[w, l])
                        hi = lo + int(ntiles[w, l])
                        if hi > lo:
                            nc.vector.tensor_tensor(
                                s[:, lo - gt0:hi - gt0, :],
                                off_sb[:, lo:hi, None].to_broadcast(
                                    [128, hi - lo, TILE]),
                                iota_sb[:, None, :].to_broadcast(
                                    [128, hi - lo, TILE]),
                                op=mybir.AluOpType.is_equal)

                zt = zpool.tile([128, GRP * WIN], F32, tag="z")
                for w in ws:
                    wtiles = []
                    for l in (0, 1):
                        t0 = int(tstart[w, l])
                        wtiles.extend(range(t0, t0 + int(ntiles[w, l])))
                    zcols = slice((w - ws[0]) * WIN, (w - ws[0] + 1) * WIN)
                    for i, t in enumerate(wtiles):
                        g, c_t0 = g_tiles[tile_chunk[t]]
                        nc.tensor.matmul(
                            zt[:, zcols],
                            lhsT=g[:, t - c_t0, :],
                            rhs=s[:, t - gt0, :],
                            start=(i == 0), stop=(i == len(wtiles) - 1))

                # flush: hT = zT * dinv_dst (bf16), then += xlT (self loop)
                gc0 = grp * GRP * WIN
                wwidth = len(ws) * WIN
                gcols = min(SHARD - gc0, wwidth)
                nc.vector.tensor_tensor(
                    hT[:, gc0:gc0 + wwidth], zt[:, :wwidth],
                    drep_sb[:, gc0:gc0 + wwidth],
                    op=mybir.AluOpType.mult)
                nc.vector.tensor_tensor(
                    hT[:, gc0:gc0 + gcols], hT[:, gc0:gc0 + gcols],
                    xlt_sb[:, gc0:gc0 + gcols],
                    op=mybir.AluOpType.add)

                # projection of this group's dest cols
                for oc in range(4):
                    op = ppool.tile([128, GRP * WIN], F32, tag="op")
                    nc.tensor.matmul(
                        op[:, :gcols],
                        lhsT=w_sb[:, oc * TILE:(oc + 1) * TILE],
                        rhs=hT[:, gc0:gc0 + gcols],
                        start=True, stop=True)
                    osb = opool.tile([128, GRP * WIN], BF16, tag="osb")
                    if has_bias:
                        nc.scalar.activation(
                            osb[:, :gcols], op[:, :gcols],
                            mybir.ActivationFunctionType.Relu,
                            bias=b_sb[:, oc:oc + 1])
                    else:
                        nc.scalar.activation(
                            osb[:, :gcols], op[:, :gcols],
                            mybir.ActivationFunctionType.Relu)
                    nc.sync.dma_start(
                        out_d[oc * TILE:(oc + 1) * TILE, gc0:gc0 + gcols],
                        osb[:, :gcols])

    nc.compile()
    return nc


_CACHE = {}


def kernel(x, edge_index, W, b):
    x = np.asarray(x, dtype=np.float32)
    W = np.asarray(W, dtype=np.float32)
    b = np.asarray(b, dtype=np.float32)
    edge_index = np.asarray(edge_index)

    per_core, layout, dinv = _prep(edge_index)
    has_bias = bool(np.any(b != 0))

    xp = np.ascontiguousarray((x * dinv[:, None]).astype(ml_dtypes.bfloat16))
    xlt = np.ascontiguousarray(
        (x * (dinv * dinv)[:, None]).astype(ml_dtypes.bfloat16).T)
    wb = np.ascontiguousarray(W.astype(ml_dtypes.bfloat16))
    iota = np.ascontiguousarray(
        np.tile(np.arange(TILE, dtype=np.float32), (128, 1)).astype(
            ml_dtypes.bfloat16))

    key = (layout["T"], tuple(layout["chunks"]), has_bias)
    if key not in _CACHE:
        _CACHE[key] = _build(layout, has_bias)
    nc = _CACHE[key]

    in_maps = []
    for c in range(NCORES):
        pc = per_core[c]
        m = dict(xp=xp, w=wb, idx=pc["idx"], off=pc["off"], iota=iota,
                 xlt=np.ascontiguousarray(
                     xlt[:, c * SHARD:(c + 1) * SHARD]),
                 drep=pc["drep"])
        if has_bias:
            m["b"] = b.reshape(F_OUT, 1)
        in_maps.append(m)

    res = run_bass_kernel_spmd(nc, in_maps, core_ids=list(range(NCORES)),
                               trace=bool(int(os.environ.get("K_TRACE", "0"))))
    kernel.last_results = res
    out = np.empty((N, F_OUT), dtype=np.float32)
    for c in range(NCORES):
        oc = res.results[c]["out"][:, :SHARD]  # [512, 6250] bf16
        out[c * SHARD:(c + 1) * SHARD] = oc.astype(np.float32).T
    return out


# revision 43
# speedup vs baseline: 1.0299x; 1.0299x over previous
"""GCN layer (gnn_message_passing) on 8 Trainium2 NeuronCores — v3.

out = relu(D^-1/2 (A+I) D^-1/2 (X W) + b),  N=50000, E=500000,
F_IN=128, F_OUT=512.

Aggregation in F_IN space: h = D^-1/2 (A+I) D^-1/2 X, out = relu(h W + b).

Per-core (dest shard of 6250 nodes, WIN=128 dest windows, 4-window PSUM
groups of 512 dests):
  1. dma_gather of x' rows (x' = X * dinv, bf16, 256B rows) for this
     core's REAL edges (self-loops excluded — handled densely) in
     8-tile/1024-desc calls (HW ring cap; more faults) load-balanced over
     4 SWDGE queues — the gather ucode runs on Q7 core-pair
     cpu_id/2 == queue_num, so 4 queues generate descriptors on 4 pairs
     concurrently. Chunks span whole (group, list) runs (fewer calls —
     per-call overhead dominates, ~2.2us/call pacing). num_idxs_reg goes
     through shared per-value Pool registers (a fresh to_reg per call
     emits a MOVE whose WAR serializes all gathers).
     src >= 32768 gathered from a base-shifted table (int16 idx limit).
  2. One-hot scatter matrices S built ON-CHIP by DVE: one
     tensor_tensor(is_equal) per (window, list) tile-range comparing the
     per-edge dest-offset stream (bf16, 255 = pad) against an iota row
     via broadcast APs — fine granularity so each window's matmuls only
     wait on their own slice.
  3. Scatter: per 128-edge tile one PE matmul zT[:, wslice] += G.T @ S.
  4. Flush per 4-window group: DVE hT = zT * dinv_dst, then
     hT += xlT (xl = X * dinv^2, the self-loop term) -> bf16.
  5. Projection per group: 4 matmuls W_chunk.T @ hT_slice -> psum,
     ACT relu(+bias) -> bf16 out tile [128, 512], DMA to out [512, 6250].
Host: pack x'/xl, idx/off streams, un-transpose + fp32-cast the output.
"""
import os
import sys

for _p in ("/opt/trn_rl_repo",):
    if _p not in sys.path and os.path.isdir(_p):
        sys.path.insert(0, _p)

import numpy as np
import ml_dtypes

import concourse.bacc as bacc
import concourse.tile as tile
from concourse import mybir
from concourse.bass_utils import run_bass_kernel_spmd
from concourse.library_config import mlp as mlp_library

N = 50000
E = 500000
F_IN = 128
F_OUT = 512
NCORES = 8
SHARD = N // NCORES            # 6250
WIN = 128
NWIN = (SHARD + WIN - 1) // WIN        # 49
GRP = 4                                 # windows per PSUM group
NGRP = (NWIN + GRP - 1) // GRP          # 13
SPLIT = 32768
TILE = 128
GT = int(os.environ.get("K_GT", "8"))   # tiles per gather call (HW ring
                                        # cap: >8 tiles = >1024 descs faults)
NQ = int(os.environ.get("K_NQ", "4"))   # SWDGE queues (4 core-pairs)
PAD = -1 if int(os.environ.get("K_PADNEG", "1")) else 0
# chunking: 'run' = chunks span a whole (group, list) run (fewer, larger
# gather calls; all pads are valid idx-0 fills), 'win' = chunks within
# window sub-runs (more calls; -1 trimmed tails)
CHUNK = os.environ.get("K_CHUNK", "run")
F32 = mybir.dt.float32
BF16 = mybir.dt.bfloat16
I16 = mybir.dt.int16
GW = NGRP * GRP * WIN  # padded dest cols (6656)


def _prep(edge_index):
    src = np.asarray(edge_index[0], dtype=np.int64)
    dst = np.asarray(edge_index[1], dtype=np.int64)

    deg = np.bincount(dst, minlength=N).astype(np.float64) + 1.0
    dinv = (1.0 / np.sqrt(deg)).astype(np.float32)

    core = dst // SHARD
    local = dst - core * SHARD
    win = local // WIN
    off = local - win * WIN
    isB = (src >= SPLIT).astype(np.int64)

    key = (core * NWIN + win) * 2 + isB
    counts = np.bincount(key, minlength=NCORES * NWIN * 2).reshape(
        NCORES, NWIN, 2)
    ntiles = (-(-counts // TILE)).max(axis=0)  # [NWIN, 2]
    # guard: every window needs >=1 tile so its psum cols get written
    empty = (ntiles[:, 0] + ntiles[:, 1]) == 0
    ntiles[empty, 0] = 1

    # tile stream: per 4-window group: A tiles of all its windows, then B.
    # Gather chunks NEVER span window sub-runs: pad slots (idx=-1) sit at
    # the end of each window's sub-run, and the ucode only trims trailing
    # negatives within a call — a mid-call -1 would be an OOB address.
    cmax = np.maximum(counts.max(axis=0), 1)  # [NWIN, 2] worst-core edges
    tstart = np.zeros((NWIN, 2), dtype=np.int64)
    pos = 0
    chunks = []              # (l, t0, ntile, q, nidx)
    qload = [0] * NQ
    vends = np.zeros((NWIN, 2), dtype=np.int64)
    for g in range(NGRP):
        ws = range(g * GRP, min((g + 1) * GRP, NWIN))
        for l in (0, 1):
            if CHUNK == "run":
                run0 = pos
                for w in ws:
                    tstart[w, l] = pos
                    vends[w, l] = int(ntiles[w, l]) * TILE  # all-valid pads
                    pos += int(ntiles[w, l])
                run1 = pos
                t = run0
                while t < run1:
                    take = min(GT, run1 - t)
                    nidx = take * TILE
                    q = min(range(NQ), key=lambda i: qload[i])
                    qload[q] += nidx + 8
                    chunks.append((l, t, take, q, nidx))
                    t += take
            else:
                for w in ws:
                    tstart[w, l] = pos
                    run0, run1 = pos, pos + int(ntiles[w, l])
                    pos = run1
                    # valid-idx end bucketed to 64 (few distinct
                    # num_idxs_reg values -> shared registers); -1 beyond
                    vend = min(int(ntiles[w, l]) * TILE,
                               -(-int(cmax[w, l]) // 64) * 64)
                    vends[w, l] = vend
                    t = run0
                    while t < run1:
                        take = min(GT, run1 - t)
                        lo = (t - run0) * TILE
                        hi = min((t - run0 + take) * TILE, vend)
                        nidx = max(hi - lo, 64)
                        q = min(range(NQ), key=lambda i: qload[i])
                        qload[q] += nidx + 8
                        chunks.append((l, t, take, q, nidx))
                        t += take
    T = pos

    order = np.lexsort((isB, win, core))
    src_s = src[order]
    off_s, isB_s, core_s, win_s = (off[order], isB[order], core[order],
                                   win[order])

    per_core = []
    for c in range(NCORES):
        sel = core_s == c
        csrc, coff = src_s[sel], off_s[sel]
        idx_flat = np.full(T * TILE, PAD, dtype=np.int64)
        off_arr = np.full((TILE, T), 255.0, dtype=np.float32)
        start = 0
        for w in range(NWIN):
            for l in (0, 1):
                cnt = counts[c, w, l]
                s0 = tstart[w, l] * TILE
                sl = slice(start, start + cnt)
                idx_flat[s0:s0 + cnt] = csrc[sl] - (SPLIT if l else 0)
                # filler (valid idx 0) up to the valid-idx end so
                # count_nonzero(idx >= 0) matches num_idxs_reg per core
                idx_flat[s0 + cnt:s0 + int(vends[w, l])] = 0
                flat = s0 + np.arange(cnt)
                off_arr[flat % TILE, flat // TILE] = coff[sl]
                start += cnt

        idx_w = np.tile(idx_flat.reshape(-1, 16).T.astype(np.int16), (8, 1))
        drep = np.zeros((128, GW), dtype=np.float32)
        drep[:, :SHARD] = np.tile(dinv[c * SHARD:(c + 1) * SHARD], (128, 1))
        per_core.append(dict(
            idx=np.ascontiguousarray(idx_w),
            off=np.ascontiguousarray(off_arr.astype(ml_dtypes.bfloat16)),
            drep=np.ascontiguousarray(drep.astype(ml_dtypes.bfloat16))))

    layout = dict(T=T, chunks=chunks, ntiles=ntiles, tstart=tstart)
    return per_core, layout, dinv


def _build(layout, has_bias):
    T = layout["T"]
    chunks = layout["chunks"]
    ntiles = layout["ntiles"]
    tstart = layout["tstart"]

    nc = bacc.Bacc("TRN2", target_bir_lowering=False, debug=False,
                   num_swdge_queues=NQ)
    x_d = nc.dram_tensor("xp", [N, F_IN], BF16, kind="ExternalInput")
    w_d = nc.dram_tensor("w", [F_IN, F_OUT], BF16, kind="ExternalInput")
    idx_d = nc.dram_tensor("idx", [128, T * 8], I16, kind="ExternalInput")
    off_d = nc.dram_tensor("off", [128, T], BF16, kind="ExternalInput")
    iota_d = nc.dram_tensor("iota", [128, TILE], BF16, kind="ExternalInput")
    xlt_d = nc.dram_tensor("xlt", [128, SHARD], BF16, kind="ExternalInput")
    drep_d = nc.dram_tensor("drep", [128, GW], BF16, kind="ExternalInput")
    if has_bias:
        b_d = nc.dram_tensor("b", [128, 4], F32, kind="ExternalInput")
    out_d = nc.dram_tensor("out", [F_OUT, GW], BF16, kind="ExternalOutput")

    # tile -> chunk id
    tile_chunk = np.zeros(T, dtype=np.int64)
    for ci, (_, t0, nt, _, _) in enumerate(chunks):
        tile_chunk[t0:t0 + nt] = ci

    with tile.TileContext(nc) as tc:
        # max tiles / chunks in any 4-window group
        gmax = 0
        for g in range(NGRP):
            ws = range(g * GRP, min((g + 1) * GRP, NWIN))
            gmax = max(gmax, sum(int(ntiles[w, l]) for w in ws
                                 for l in (0, 1)))
        cpg = np.zeros(NGRP, dtype=np.int64)
        for ci, (_, t0, nt, _, _) in enumerate(chunks):
            for g in range(NGRP):
                ws = range(g * GRP, min((g + 1) * GRP, NWIN))
                lo = int(tstart[ws[0], 0])
                hi = int(tstart[ws[-1], 1] + ntiles[ws[-1], 1])
                if lo <= t0 < hi:
                    cpg[g] += 1
                    break
        gbufs = int(os.environ.get(
            "K_GBUFS", int(cpg.max()) + max(6, int(cpg.max()))))

        with (
            tc.tile_pool(name="const", bufs=1) as cpool,
            tc.tile_pool(name="g", bufs=gbufs) as gpool,
            tc.tile_pool(name="sbuf_s", bufs=4) as spool,
            tc.tile_pool(name="osb", bufs=4) as opool,
            tc.tile_pool(name="zps", bufs=5, space="PSUM") as zpool,
            tc.tile_pool(name="pps", bufs=3, space="PSUM") as ppool,
        ):
            nc.gpsimd.load_library(mlp_library)

            # idx split: group 0's slice lands first so the first gathers
            # don't wait for the whole 1.1MB table
            tsplit = int(tstart[GRP, 0]) if NGRP > 1 else T
            idx_a = cpool.tile([128, tsplit * 8], I16)
            nc.sync.dma_start(idx_a[:], idx_d[:, :tsplit * 8])
            off_sb = cpool.tile([128, T], BF16)
            nc.sync.dma_start(off_sb[:], off_d[:])
            iota_sb = cpool.tile([128, TILE], BF16)
            nc.sync.dma_start(iota_sb[:], iota_d[:])
            idx_b = cpool.tile([128, (T - tsplit) * 8], I16)
            nc.sync.dma_start(idx_b[:], idx_d[:, tsplit * 8:])
            w_sb = cpool.tile([128, F_OUT], BF16)
            nc.sync.dma_start(w_sb[:], w_d[:])
            xlt_sb = cpool.tile([128, GW], BF16)
            nc.sync.dma_start(xlt_sb[:, :SHARD], xlt_d[:])
            # drep uploaded as bf16 (half the bytes) and widened to fp32
            # on-chip with one DVE copy
            drep_h = cpool.tile([128, GW], BF16)
            nc.sync.dma_start(drep_h[:], drep_d[:])
            drep_sb = cpool.tile([128, GW], F32)
            nc.vector.tensor_copy(drep_sb[:], drep_h[:])
            hT = cpool.tile([128, GW], BF16)
            if has_bias:
                b_sb = cpool.tile([128, 4], F32)
                nc.sync.dma_start(b_sb[:], b_d[:])

            x_lo = x_d[:SPLIT, :]
            x_hi = x_d[SPLIT:, :]

            # one-time zero of every gather buffer: trimmed (-1) idx tail
            # slots are never written by the DMA, and NaN garbage would
            # poison the matmul (0 * NaN = NaN) despite zero S columns.
            for _ in range(gbufs):
                tmp = gpool.tile([128, GT, F_IN], BF16, tag="g")
                nc.vector.memset(tmp[:], 0)

            g_tiles = {}

            # one shared Pool register per distinct idx count — a fresh
            # to_reg per gather emits a MOVE whose WAR hazard serializes
            # every gather call behind the previous one's retirement.
            nidx_regs = {v: nc.gpsimd.to_reg(v)
                         for v in sorted({c[4] for c in chunks})}

            def gather_chunk(ci):
                l, t0, nt, q, nidx = chunks[ci]
                g = gpool.tile([128, GT, F_IN], BF16, tag="g")
                if t0 >= tsplit:
                    isb = idx_b[:, (t0 - tsplit) * 8:(t0 - tsplit + nt) * 8]
                else:
                    isb = idx_a[:, t0 * 8:(t0 + nt) * 8]
                nc.gpsimd.dma_gather(
                    g[:, :nt, :],
                    (x_hi if l else x_lo),
                    isb,
                    num_idxs=nt * TILE,
                    num_idxs_reg=nidx_regs[nidx],
                    elem_size=F_IN,
                    queue_num=q,
                )
                g_tiles[ci] = (g, t0)

            next_chunk = [0]

            def gather_upto(tile_end):
                while (next_chunk[0] < len(chunks)
                       and chunks[next_chunk[0]][1] < tile_end):
                    gather_chunk(next_chunk[0])
                    next_chunk[0] += 1

            for grp in range(NGRP):
                ws = list(range(grp * GRP, min((grp + 1) * GRP, NWIN)))
                wlast = ws[-1]
                gt0 = int(tstart[ws[0], 0])
                tile_end = int(tstart[wlast, 1] + ntiles[wlast, 1])
                nw = tile_end - gt0
                gather_upto(tile_end)

                # on-chip one-hot build: S[p, t, c] = (off[p, t] == c).
                # Two ops (A-list tiles, then B) so A matmuls only wait on
                # the A build.
                s = spool.tile([128, gmax, TILE], BF16, tag="s")
                for l in (0, 1):
                    for w in ws:
                        lo = int(tstart[w, l])
                        hi = lo + int(ntiles[w, l])
                        if hi > lo:
                            nc.vector.tensor_tensor(
                                s[:, lo - gt0:hi - gt0, :],
                                off_sb[:, lo:hi, None].to_broadcast(
                                    [128, hi - lo, TILE]),
                                iota_sb[:, None, :].to_broadcast(
                                    [128, hi - lo, TILE]),
                                op=mybir.AluOpType.is_equal)

                zt = zpool.tile([128, GRP * WIN], F32, tag="z")
                for w in ws:
                    wtiles = []
                    for l in (0, 1):
                        t0 = int(tstart[w, l])
                        wtiles.extend(range(t0, t0 + int(ntiles[w, l])))
                    zcols = slice((w - ws[0]) * WIN, (w - ws[0] + 1) * WIN)
                    for i, t in enumerate(wtiles):
                        g, c_t0 = g_tiles[tile_chunk[t]]
                        nc.tensor.matmul(
                            zt[:, zcols],
                            lhsT=g[:, t - c_t0, :],
                            rhs=s[:, t - gt0, :],
                            start=(i == 0), stop=(i == len(wtiles) - 1))

                # flush: hT = zT * dinv_dst (bf16), then += xlT (self loop)
                gc0 = grp * GRP * WIN
                wwidth = len(ws) * WIN
                gcols = min(SHARD - gc0, wwidth)
                nc.vector.tensor_tensor(
                    hT[:, gc0:gc0 + wwidth], zt[:, :wwidth],
                    drep_sb[:, gc0:gc0 + wwidth],
                    op=mybir.AluOpType.mult)
                nc.vector.tensor_tensor(
                    hT[:, gc0:gc0 + gcols], hT[:, gc0:gc0 + gcols],
                    xlt_sb[:, gc0:gc0 + gcols],
                    op=mybir.AluOpType.add)

                # projection of this group's dest cols
                for oc in range(4):
                    op = ppool.tile([128, GRP * WIN], F32, tag="op")
                    nc.tensor.matmul(
                        op[:, :gcols],
                        lhsT=w_sb[:, oc * TILE:(oc + 1) * TILE],
                        rhs=hT[:, gc0:gc0 + gcols],
                        start=True, stop=True)
                    osb = opool.tile([128, GRP * WIN], BF16, tag="osb")
                    if has_bias:
                        nc.scalar.activation(
                            osb[:, :gcols], op[:, :gcols],
                            mybir.ActivationFunctionType.Relu,
                            bias=b_sb[:, oc:oc + 1])
                    else:
                        nc.scalar.activation(
                            osb[:, :gcols], op[:, :gcols],
                            mybir.ActivationFunctionType.Relu)
                    nc.sync.dma_start(
                        out_d[oc * TILE:(oc + 1) * TILE, gc0:gc0 + gcols],
                        osb[:, :gcols])

    nc.compile()
    return nc


_CACHE = {}


def kernel(x, edge_index, W, b):
    x = np.asarray(x, dtype=np.float32)
    W = np.asarray(W, dtype=np.float32)
    b = np.asarray(b, dtype=np.float32)
    edge_index = np.asarray(edge_index)

    per_core, layout, dinv = _prep(edge_index)
    has_bias = bool(np.any(b != 0))

    xp = np.ascontiguousarray((x * dinv[:, None]).astype(ml_dtypes.bfloat16))
    xlt = np.ascontiguousarray(
        (x * (dinv * dinv)[:, None]).astype(ml_dtypes.bfloat16).T)
    wb = np.ascontiguousarray(W.astype(ml_dtypes.bfloat16))
    iota = np.ascontiguousarray(
        np.tile(np.arange(TILE, dtype=np.float32), (128, 1)).astype(
            ml_dtypes.bfloat16))

    key = (layout["T"], tuple(layout["chunks"]), has_bias)
    if key not in _CACHE:
        _CACHE[key] = _build(layout, has_bias)
    nc = _CACHE[key]

    in_maps = []
    for c in range(NCORES):
        pc = per_core[c]
        m = dict(xp=xp, w=wb, idx=pc["idx"], off=pc["off"], iota=iota,
                 xlt=np.ascontiguousarray(
                     xlt[:, c * SHARD:(c + 1) * SHARD]),
                 drep=pc["drep"])
        if has_bias:
            m["b"] = b.reshape(F_OUT, 1)
        in_maps.append(m)

    res = run_bass_kernel_spmd(nc, in_maps, core_ids=list(range(NCORES)),
                               trace=bool(int(os.environ.get("K_TRACE", "0"))))
    kernel.last_results = res
    out = np.empty((N, F_OUT), dtype=np.float32)
    for c in range(NCORES):
        oc = res.results[c]["out"][:, :SHARD]  # [512, 6250] bf16
        out[c * SHARD:(c + 1) * SHARD] = oc.astype(np.float32).T
    return out
